# revision 55
# baseline (speedup 1.0000x reference)
"""Trainium2 Bass kernel for nn_MultiHeadAttention_3590592660317.

Sequence-sharded across 8 cores (512 rows each); each core redundantly
computes full K/V (cheap) plus its own Q rows, attention, and output
rows -> no collectives; the host concatenates the 8 output slices.

Schedule (v2): the ACT engine's 128 exps (~131us) are the floor; every
other engine is arranged around keeping ACT fed.
  - prep (~12us): weights, own-row + window-0 norms (ACT Square +
    Ln/Exp(-0.5) rsqrt, all in the natural_log_exp table set so the exp
    stream never reloads tables), qT (both groups), kT g0 w0, v 0-3.
  - build phase: pairs 0 AND 1 (both head-group 0) stream attention
    together over windows as they are built (8 exps/window), with
    window w+1's norm (gpsimd squares), scale+transpose (sync DGE), kT
    g0 and v chunks interleaved between blocks.  Pair 1's PV for
    kc>=22 is deferred (pts parked in SBUF) to offload the build-phase
    tensor engine.
  - phase 2: pairs 2,3 run back-to-back; kT g1 windows are built
    just-in-time under pair 2's exp stream, pair 1's deferred PV
    batches drain here, then pair flushes/norms and the output
    projection (bias pre-folded into the residual rows).
"""

import sys

if "/opt/trn_rl_repo" not in sys.path:
    sys.path.insert(0, "/opt/trn_rl_repo")

import numpy as np

import concourse.bass as bass
import concourse.tile as tile
from concourse import mybir
from concourse.bass_utils import run_bass_kernel_spmd
from concourse.tile import add_dep_helper

F32 = mybir.dt.float32
BF16 = mybir.dt.bfloat16
I32 = mybir.dt.int32

N = 4096
IN_DIM = 256
H = 8
HD = 32
DIN = 259
NC = 8
R = N // NC  # 512 rows per core

_KDIMS = (128, 128, 4)  # contraction chunks: 256 feats + (3 coords + ones)
DEFER_FROM = 11  # pair1 PV batches for pts-block kg >= this drain in phase 2


def _split_oversized_waits(nc, max_waits=1):
    """This walrus build only encodes one sync-wait per instruction; move
    excess on_wait conditions onto preceding same-engine NOPs."""
    nsplit = 0
    for f in nc.m.functions:
        for b in f.blocks:
            new_instrs = []
            for ins in b.instructions:
                si = getattr(ins, "sync_info", None)
                waits = list(si.on_wait) if si is not None and si.on_wait else []
                changed = False
                while len(waits) > max_waits:
                    chunk, waits = waits[:max_waits], waits[max_waits:]
                    nop = mybir.InstNoOp(name=f"{ins.name}-ws{nsplit}", ins=[], outs=[])
                    nop.engine = ins.engine
                    nop.sync_info = mybir.SyncInfo(on_wait=chunk, on_update=[])
                    new_instrs.append(nop)
                    nsplit += 1
                    changed = True
                if changed:
                    ins.sync_info = mybir.SyncInfo(
                        on_wait=waits, on_update=list(si.on_update or [])
                    )
                new_instrs.append(ins)
            b.instructions = new_instrs
    return nsplit


def build_nc(split=True):
    nc = bass.Bass()

    x = nc.dram_tensor("x", [N, IN_DIM], F32, kind="ExternalInput")
    coordsT = nc.dram_tensor("coordsT", [3, N], I32, kind="ExternalInput")
    xq = nc.dram_tensor("xq", [R, IN_DIM], F32, kind="ExternalInput")
    cqT = nc.dram_tensor("cqT", [3, R], I32, kind="ExternalInput")
    wq = nc.dram_tensor("wq", [H, DIN, HD], F32, kind="ExternalInput")
    bq = nc.dram_tensor("bq", [H, HD], F32, kind="ExternalInput")
    wk = nc.dram_tensor("wk", [H, DIN, HD], F32, kind="ExternalInput")
    bk = nc.dram_tensor("bk", [H, HD], F32, kind="ExternalInput")
    wv = nc.dram_tensor("wv", [H, DIN, HD], F32, kind="ExternalInput")
    bv = nc.dram_tensor("bv", [H, HD], F32, kind="ExternalInput")
    wo = nc.dram_tensor("wo", [IN_DIM, IN_DIM], F32, kind="ExternalInput")
    bo = nc.dram_tensor("bo", [IN_DIM], F32, kind="ExternalInput")
    out = nc.dram_tensor("out", [R, IN_DIM], F32, kind="ExternalOutput")

    with tile.TileContext(nc) as tc:
        _body(tc, nc, x, coordsT, xq, cqT, wq, bq, wk, bk, wv, bv, wo, bo, out)

    if split:
        _split_oversized_waits(nc)
    return nc


def _body(tc, nc, x, coordsT, xq, cqT, wq, bq, wk, bk, wv, bv, wo, bo, out):
    from contextlib import ExitStack

    ctx = ExitStack()
    with ctx:
        sing = ctx.enter_context(tc.tile_pool(name="sing", bufs=1))
        stream = ctx.enter_context(tc.tile_pool(name="stream", bufs=2))
        ftp = ctx.enter_context(tc.tile_pool(name="ftp", bufs=3))
        sqp = ctx.enter_context(tc.tile_pool(name="sqp", bufs=2))
        wstg = ctx.enter_context(tc.tile_pool(name="wstg", bufs=2))
        ptp = ctx.enter_context(tc.tile_pool(name="ptp", bufs=10))
        # deferred pair-1 pts live here (one-shot tiles, no rotation pressure)
        dfp = ctx.enter_context(tc.tile_pool(name="dfp", bufs=10))
        psA = ctx.enter_context(tc.tile_pool(name="psA", bufs=3, space="PSUM"))
        psB = ctx.enter_context(tc.tile_pool(name="psB", bufs=2, space="PSUM"))

        # ---------------- persistent SBUF tiles ----------------
        xcT0 = sing.tile([128, N], BF16, tag="xcT0", name="xcT0")
        xcT1 = sing.tile([128, N], BF16, tag="xcT1", name="xcT1")
        cnT = sing.tile([4, N], BF16, tag="cnT", name="cnT")  # coords + ones
        xcq0 = sing.tile([128, R], BF16, tag="xcq0", name="xcq0")
        xcq1 = sing.tile([128, R], BF16, tag="xcq1", name="xcq1")
        cnq = sing.tile([4, R], BF16, tag="cnq", name="cnq")
        kT = [sing.tile([128, N], BF16, tag=f"kT{g}", name=f"kT{g}") for g in range(2)]
        qT = [sing.tile([128, R], BF16, tag=f"qT{g}", name=f"qT{g}") for g in range(2)]
        # halves-swapped copies: head h%4 also lives at strip (h%4+2)%4
        kTa = [sing.tile([128, N], BF16, tag=f"kTa{g}", name=f"kTa{g}") for g in range(2)]
        qTa = [sing.tile([128, R], BF16, tag=f"qTa{g}", name=f"qTa{g}") for g in range(2)]
        # v natural layout with a ones column per head: [128, rc, 33*8]
        v_sb = sing.tile([128, N // 128, 33 * H], BF16, tag="vv", name="vv")
        catT = [sing.tile([128, R], BF16, tag=f"catT{g}", name=f"catT{g}") for g in range(2)]
        # x windows rotate through 4 slots (DMA -> squares -> scale spans 3)
        x_all = sing.tile([128, 4, 4, IN_DIM], F32, tag="x_all", name="x_all")
        xq_all = sing.tile([128, R // 128, IN_DIM], F32, tag="xq_all", name="xq_all")
        ss_all = sing.tile([128, 36], F32, tag="ss_all", name="ss_all")
        inv_all = sing.tile([128, 36], F32, tag="inv_all", name="inv_all")
        cq_f_g = sing.tile([4, R], F32, tag="cq_f_g", name="cq_f_g")

        # weights (bf16); chunk 2 is [3 coord rows + bias row]
        wk_sb = [sing.tile([_KDIMS[c], H, HD], BF16, tag=f"wk{c}", name=f"wk{c}") for c in range(3)]
        wq_sb = [sing.tile([_KDIMS[c], H, HD], BF16, tag=f"wq{c}", name=f"wq{c}") for c in range(3)]
        wv_sb = [sing.tile([_KDIMS[c], H * HD], BF16, tag=f"wv{c}", name=f"wv{c}") for c in range(3)]
        wo_sb = [sing.tile([128, IN_DIM], BF16, tag=f"wo{c}", name=f"wo{c}") for c in range(2)]
        bo_f32 = sing.tile([1, IN_DIM], F32, tag="bo32", name="bo32")
        ones_colf = sing.tile([1, 128], F32, tag="ones_colf", name="ones_colf")
        # block-select ones [64, 64]: row 0 -> out rows 0..31, row 32 -> out
        # rows 32..63 (engine partition offsets must be 32-aligned)
        ones2 = sing.tile([2 * HD, 2 * HD], F32, tag="ones2", name="ones2")
        bo_bc = sing.tile([128, IN_DIM], F32, tag="bo_bc", name="bo_bc")

        nc.vector.memset(ones_colf, 1.0)
        nc.vector.memset(ones2, 0.0)
        nc.vector.memset(ones2[0:1, 0:HD], 1.0)
        nc.vector.memset(ones2[HD:HD + 1, HD:2 * HD], 1.0)

        # ---------------- weight loads + casts ----------------
        def load_weights_main():
            # q/k/v projection weights + coords: everything the prep and
            # build phases need.  wo/bo load later (only the tail uses them).
            # coords + the c-chunk DMAs go first on their queues: the c=2
            # (coords/bias) matmul chunk gates every projection.
            # stage coords as [128,128]: rows 0-95 = 3 coord rows x 32 blocks,
            # rows 96-127 = 1.0 so the clamp+writeback also produces the ones
            # row of cnT (bias folding) without a wide memset
            nc.vector.memset(cq_f_g, 1.0)
            nc.gpsimd.dma_start(
                out=cq_f_g.bitcast(I32)[0:3], in_=cqT[:, :]
            )
            cwide = wstg.tile([128, 128], F32, tag="cwide", name="cwide")
            nc.vector.memset(cwide[96:128, :], 1.0)
            nc.sync.dma_start(
                out=cwide.bitcast(I32)[0:96],
                in_=coordsT[:, :].rearrange("c (j f) -> (c j) f", f=128),
            )
            stgs = {}
            for c in (2, 0, 1):
                kd = (128, 128, 3)[c]
                dsl = slice(c * 128, c * 128 + kd)
                for wi, (w_dram, b_dram) in enumerate(
                    ((wk, bk), (wq, bq), (wv, bv))
                ):
                    stg = sing.tile(
                        [128 if c != 2 else 4, H, HD], F32,
                        tag=f"stg{c}{wi}", name=f"stg{c}{wi}",
                    )
                    weng = (nc.gpsimd, nc.sync, nc.gpsimd)[c]
                    weng.dma_start(
                        out=stg[:kd], in_=w_dram[:, dsl, :].rearrange("h d k -> d h k")
                    )
                    if c == 2:
                        nc.gpsimd.dma_start(
                            out=stg[3:4],
                            in_=b_dram[:, :].rearrange("h k -> (h k)")[None, :]
                            .rearrange("a (h k) -> a h k", h=H),
                        )
                    stgs[(c, wi)] = stg
            # coord casts/clamps early on vector; c=2 weight casts (small,
            # gate the early projection chunk)
            nc.vector.tensor_copy(
                out=cwide[0:96], in_=cwide.bitcast(I32)[0:96]
            )
            cwb = wstg.tile([128, 128], BF16, tag="cwb", name="cwb")
            nc.vector.tensor_scalar_min(out=cwb, in0=cwide, scalar1=100.0)
            nc.sync.dma_start(out=cnT, in_=cwb)
            nc.vector.tensor_copy(out=cq_f_g[0:3], in_=cq_f_g.bitcast(I32)[0:3])
            nc.vector.tensor_scalar_min(out=cnq, in0=cq_f_g, scalar1=100.0)
            for wi, w_tile in enumerate((wk_sb[2], wq_sb[2], wv_sb[2])):
                w3 = (
                    w_tile if w_tile.shape[1] == H
                    else w_tile.rearrange("d (h k) -> d h k", h=H)
                )
                nc.vector.tensor_copy(out=w3[:4], in_=stgs[(2, wi)][:4])
            return stgs

        def load_weights_casts(stgs):
            # staging tiles are one-shot (distinct tags) so deferring these
            # casts is safe; emitted after the critical norm chain
            for c in (0, 1):
                for wi, w_tile in enumerate((wk_sb[c], wq_sb[c], wv_sb[c])):
                    w3 = (
                        w_tile if w_tile.shape[1] == H
                        else w_tile.rearrange("d (h k) -> d h k", h=H)
                    )
                    nc.vector.tensor_copy(out=w3[:128], in_=stgs[(c, wi)][:128])
            # ones columns of v (col 32 of each head's 33-wide block)
            v4 = v_sb.rearrange("p r (h c) -> p r h c", c=33)
            nc.vector.memset(v4[:, :, :, 32:33], 1.0)

        def load_weights_tail():
            for c in range(2):
                stg = wstg.tile([128, IN_DIM], F32, tag="wstg2", name="wstg2")
                nc.gpsimd.dma_start(out=stg, in_=wo[c * 128:(c + 1) * 128, :])
                nc.vector.tensor_copy(out=wo_sb[c], in_=stg)
            nc.gpsimd.dma_start(out=bo_f32, in_=bo[None, :])
            # broadcast bo once, then fold into the residual rows
            bct = psA.tile([128, 1024], F32, tag="att", name="bo_bc_ps")
            nc.tensor.matmul(
                bct[:, 0:IN_DIM], lhsT=ones_colf, rhs=bo_f32, start=True, stop=True
            )
            nc.vector.tensor_copy(out=bo_bc, in_=bct[:, 0:IN_DIM])
            for rw in range(4):
                nc.vector.tensor_add(
                    out=xq_all[:, rw, :], in0=xq_all[:, rw, :], in1=bo_bc
                )

        # ---------------- norm machinery ----------------
        def dma_x_window(w, eng):
            eng.dma_start(
                out=x_all[:, w % 4],
                in_=x[w * 512:(w + 1) * 512, :].rearrange("(c p) d -> p c d", p=128),
            )

        def squares_act(xbuf, nchunk, ss_col):
            # ACT Square with accumulate: used in prep only (table co-resident
            # with Exp/Ln so the exp stream never reloads)
            for i in range(nchunk):
                sq = stream.tile([128, IN_DIM], F32, tag="sqa", name="sqa")
                nc.scalar.activation(
                    out=sq, in_=xbuf[:, i, :],
                    func=mybir.ActivationFunctionType.Square,
                    accum_out=ss_all[:, ss_col + i:ss_col + i + 1],
                )

        def squares_gp(w):
            # in-stream norms go to gpsimd so ACT only runs exps
            sq = sqp.tile([128, 4, IN_DIM], F32, tag="sqg", name="sqg")
            nc.gpsimd.tensor_mul(out=sq, in0=x_all[:, w % 4], in1=x_all[:, w % 4])
            nc.vector.tensor_reduce(
                out=ss_all[:, 4 * w:4 * w + 4], in_=sq,
                axis=mybir.AxisListType.X, op=mybir.AluOpType.add,
            )

        def inv_lnexp(col0, ncols):
            # 1/sqrt(ss) = exp(-0.5*ln(ss)); Ln and Exp share a table set
            lnb = stream.tile([128, 4], F32, tag="lnb", name="lnb")
            nc.scalar.activation(
                out=lnb, in_=ss_all[:, col0:col0 + ncols],
                func=mybir.ActivationFunctionType.Ln,
            )
            nc.scalar.activation(
                out=inv_all[:, col0:col0 + ncols], in_=lnb,
                func=mybir.ActivationFunctionType.Exp, scale=-0.5,
            )

        def scale_mul(rc, lo, hi, xap, ss_col):
            # split the scaled chunk into window-combined lo/hi tiles so one
            # batched xbar transpose per half covers the whole 512-row window
            inv1 = inv_all[:, ss_col:ss_col + 1]
            nc.vector.tensor_scalar_mul(
                out=lo[:, rc % 4, :], in0=xap[:, 0:128], scalar1=inv1
            )
            nc.vector.tensor_scalar_mul(
                out=hi[:, rc % 4, :], in0=xap[:, 128:256], scalar1=inv1
            )

        def scale_transpose(nw, t0, t1, lo, hi, eng0=None, eng1=None):
            nsl = slice(nw * 512, (nw + 1) * 512)
            (eng0 or nc.sync).dma_start_transpose(
                out=t0[:, nsl].rearrange("f (g r) -> f g r", r=128),
                in_=lo.rearrange("p g r -> p (g r)"),
            )
            (eng1 or nc.sync).dma_start_transpose(
                out=t1[:, nsl].rearrange("f (g r) -> f g r", r=128),
                in_=hi.rearrange("p g r -> p (g r)"),
            )

        def xcT_chunk(c, full=True):
            if full:
                return (xcT0, xcT1, cnT)[c]
            return (xcq0, xcq1, cnq)[c]

        # ---------------- projections ----------------
        def kq_window(w_sb_, dst, alt, g, nw, full=True, alt_eng=None):
            nsl = slice(nw * 512, (nw + 1) * 512)
            pst = psA.tile([128, 1024], F32, tag="att", name="proj")
            ps = pst[:, 0:512]
            # c=2 (coords+bias) first: its operands are ready earliest
            for c in (2, 0, 1):
                kd = _KDIMS[c]
                nc.tensor.matmul(
                    ps,
                    lhsT=w_sb_[c][:kd, g * 4:g * 4 + 4, :].rearrange(
                        "d h k -> d (h k)"
                    ),
                    rhs=xcT_chunk(c, full)[:kd, nsl],
                    start=(c == 2),
                    stop=(c == 1),
                    skip_group_check=True,
                )
            nc.vector.tensor_copy(out=dst[g][:, nsl], in_=ps)
            if alt is not None:
                ae = alt_eng or nc.vector
                ae.tensor_copy(out=alt[g][0:64, nsl], in_=ps[64:128, :])
                ae.tensor_copy(out=alt[g][64:128, nsl], in_=ps[0:64, :])

        def v_chunk(rc):
            rsl = slice(rc * 128, (rc + 1) * 128)
            pst = psA.tile([128, 1024], F32, tag="att", name="projv")
            ps = pst[:, 0:H * HD]
            for c in (2, 0, 1):
                kd = _KDIMS[c]
                nc.tensor.matmul(
                    ps,
                    lhsT=xcT_chunk(c)[:kd, rsl],
                    rhs=wv_sb[c][:kd, :],
                    start=(c == 2),
                    stop=(c == 1),
                    skip_group_check=True,
                )
            nc.vector.tensor_copy(
                out=v_sb[:, rc, :].rearrange("p (h k) -> p h k", h=H)[:, :, 0:HD],
                in_=ps.rearrange("p (h k) -> p h k", h=H),
            )

        # ---------------- attention ----------------
        SCALE = 1.0 / float(np.sqrt(HD))

        class _St:
            pass

        def attn_begin(pair, defer_from=None):
            st = _St()
            st.pair = pair
            st.g = pair // 2
            st.hp0 = (pair % 2) * 2
            pvt = psB.tile([128, 512], F32, tag="pv", name="pv")
            st.pvps = pvt[0:97, :]
            st.prev_pts = None
            st.prev_kg = None
            st.pv_last = None
            st.defer_from = defer_from
            st.deferred = []
            return st

        def _pv_batch(st, kgp, pts, qk_last=None):
            for sub in range(2):
                kc = 2 * kgp + sub
                for a in range(2):
                    h = 2 * st.pair + a
                    mm = nc.tensor.matmul(
                        st.pvps[64 * a:64 * a + 33, :],
                        lhsT=v_sb[:, kc, 33 * h:33 * h + 33],
                        rhs=pts[a][:, sub * 512:(sub + 1) * 512],
                        start=(kc == 0),
                        stop=(kc == N // 128 - 1),
                        tile_position=(0, 64 * a),
                        skip_group_check=True,
                    )
                    if qk_last is not None:
                        add_dep_helper(mm.ins, qk_last.ins, sync=False)
                    st.pv_last = mm

        def attn_block(st, kg, no_alt=False):
            g, hp0 = st.g, st.hp0
            qk_last = None
            pts = None
            if kg < 16:
                attps = [
                    psA.tile([128, 1024], F32, tag="att", name="att")
                    for _ in range(2)
                ]
                for sub in range(2):
                    kc = 2 * kg + sub
                    ksl = slice(kc * 128, (kc + 1) * 128)
                    par = (kc % 2) if not no_alt else 0
                    kt_src = (kT, kTa)[par]
                    qt_src = (qT, qTa)[par]
                    for a in range(2):
                        s = (hp0 + a + 2 * par) % 4
                        mm = nc.tensor.matmul(
                            attps[a][:, sub * 512:(sub + 1) * 512],
                            lhsT=kt_src[g][32 * s:32 * s + 32, ksl],
                            rhs=qt_src[g][32 * s:32 * s + 32, :],
                            start=True,
                            stop=True,
                            tile_position=(32 * s, 0),
                        )
                        if st.pv_last is not None:
                            add_dep_helper(mm.ins, st.pv_last.ins, sync=False)
                        qk_last = mm
                pts = []
                will_defer = st.defer_from is not None and kg >= st.defer_from
                pool = dfp if will_defer else ptp
                for a in range(2):
                    pt = pool.tile([128, 1024], BF16, tag="pt", name="pt")
                    nc.scalar.activation(
                        out=pt,
                        in_=attps[a],
                        func=mybir.ActivationFunctionType.Exp,
                        scale=SCALE,
                    )
                    pts.append(pt)
            if st.prev_pts is not None:
                if st.defer_from is not None and st.prev_kg >= st.defer_from:
                    st.deferred.append((st.prev_kg, st.prev_pts))
                else:
                    _pv_batch(st, st.prev_kg, st.prev_pts, qk_last)
            st.prev_pts = pts
            st.prev_kg = kg if kg < 16 else None

        def attn_flush(st):
            attn_block(st, 16)

        def drain_deferred(st, nbatch=1):
            for _ in range(nbatch):
                if st.deferred:
                    kgp, pts = st.deferred.pop(0)
                    _pv_batch(st, kgp, pts)

        def attn_norm(st, act_recip=False):
            g, hp0 = st.g, st.hp0
            # sums land at rows 0 and 32 (32-aligned); rows in between hold
            # 1.0 so reciprocal stays finite (they hit zero lhsT weights)
            s_sb = stream.tile([2 * HD, 512], F32, tag="s_sb", name="s_sb")
            nc.gpsimd.memset(s_sb, 1.0)
            for a in range(2):
                nc.vector.tensor_scalar_add(
                    out=s_sb[HD * a:HD * a + 1, :],
                    in0=st.pvps[64 * a + 32:64 * a + 33, :],
                    scalar1=1e-6,
                )
            r_sb = stream.tile([2 * HD, 512], F32, tag="r_sb", name="r_sb")
            if act_recip:
                # tail only (ACT idle there): 1/s = exp(-ln(s)), same table
                # set as the exp stream
                lnr = stream.tile([2 * HD, 512], F32, tag="lnr", name="lnr")
                nc.scalar.activation(
                    out=lnr, in_=s_sb, func=mybir.ActivationFunctionType.Ln
                )
                nc.scalar.activation(
                    out=r_sb, in_=lnr,
                    func=mybir.ActivationFunctionType.Exp, scale=-1.0,
                )
            else:
                nc.vector.reciprocal(out=r_sb, in_=s_sb)
            rbct = psA.tile([128, 1024], F32, tag="att", name="rbc")
            rbc = rbct[0:2 * HD, 0:512]
            nc.tensor.matmul(rbc, lhsT=ones2, rhs=r_sb, start=True, stop=True)
            rbc_sb = stream.tile([2 * HD, 512], F32, tag="rbc_sb", name="rbc_sb")
            nc.vector.tensor_copy(out=rbc_sb, in_=rbc)
            for a in range(2):
                nc.vector.tensor_mul(
                    out=catT[g][32 * (hp0 + a):32 * (hp0 + a) + 32, :],
                    in0=st.pvps[64 * a:64 * a + 32, :],
                    in1=rbc_sb[32 * a:32 * a + 32, :],
                )

        # ---------------- emission ----------------
        # critical x DMAs chunked + first so norms start as rows land
        for rc in range(4):
            eng = (nc.sync, nc.gpsimd)[rc % 2]
            eng.dma_start(
                out=xq_all[:, rc, :], in_=xq[rc * 128:(rc + 1) * 128, :]
            )
        for rc in range(4):
            eng = (nc.sync, nc.gpsimd)[rc % 2]
            eng.dma_start(
                out=x_all[:, 0, rc, :], in_=x[rc * 128:(rc + 1) * 128, :]
            )
        stgs = load_weights_main()

        squares_act(xq_all, 4, 32)
        inv_lnexp(32, 4)
        squares_act(x_all[:, 0], 4, 0)
        inv_lnexp(0, 4)

        flo = ftp.tile([128, 4, 128], BF16, tag="flo", name="flo")
        fhi = ftp.tile([128, 4, 128], BF16, tag="fhi", name="fhi")
        for rc in range(4):
            scale_mul(rc, flo, fhi, xq_all[:, rc, :], 32 + rc)
        scale_transpose(0, xcq0, xcq1, flo, fhi, eng0=nc.sync, eng1=nc.sync)
        flo = ftp.tile([128, 4, 128], BF16, tag="flo", name="flo")
        fhi = ftp.tile([128, 4, 128], BF16, tag="fhi", name="fhi")
        for rc in range(4):
            scale_mul(rc, flo, fhi, x_all[:, 0, rc, :], rc)
        scale_transpose(0, xcT0, xcT1, flo, fhi, eng0=nc.scalar, eng1=nc.sync)
        load_weights_casts(stgs)
        for g in range(2):
            kq_window(wq_sb, qT, qTa, g, 0, full=False)
        dma_x_window(1, nc.gpsimd)
        dma_x_window(2, nc.gpsimd)
        kq_window(wk_sb, kT, kTa, 0, 0)
        for rc in range(4):
            v_chunk(rc)
        squares_gp(1)

        # -------- build phase: pairs 0+1 stream over windows as built -----
        st0 = attn_begin(0)
        st1 = attn_begin(1, defer_from=DEFER_FROM)
        for nw in range(8):
            wn = nw + 1
            if wn + 2 <= 7:
                dma_x_window(wn + 2, nc.gpsimd)
            if wn + 1 <= 7:
                squares_gp(wn + 1)
            if wn <= 7:
                inv_lnexp(4 * wn, 4)
                flo = ftp.tile([128, 4, 128], BF16, tag="flo", name="flo")
                fhi = ftp.tile([128, 4, 128], BF16, tag="fhi", name="fhi")
                scale_mul(0, flo, fhi, x_all[:, wn % 4, 0, :], 4 * wn + 0)
                scale_mul(1, flo, fhi, x_all[:, wn % 4, 1, :], 4 * wn + 1)
            attn_block(st0, 2 * nw)
            if wn <= 7:
                scale_mul(2, flo, fhi, x_all[:, wn % 4, 2, :], 4 * wn + 2)
                scale_mul(3, flo, fhi, x_all[:, wn % 4, 3, :], 4 * wn + 3)
                scale_transpose(wn, xcT0, xcT1, flo, fhi)
            attn_block(st1, 2 * nw)
            if wn <= 7:
                kq_window(wk_sb, kT, kTa, 0, wn)
            elif nw == 7:
                kq_window(wk_sb, kT, kTa, 1, 0)
            attn_block(st0, 2 * nw + 1)
            if wn <= 7:
                v_chunk(4 * wn + 0)
                v_chunk(4 * wn + 1)
            elif nw == 7:
                kq_window(wk_sb, kT, kTa, 1, 1)
            attn_block(st1, 2 * nw + 1)
            if wn <= 7:
                v_chunk(4 * wn + 2)
                v_chunk(4 * wn + 3)

        attn_flush(st0)
        attn_flush(st1)  # lands in the deferred list

        # -------- phase 2: pairs 2,3; kT g1 + deferred PV drain in-stream --
        st2 = attn_begin(2)
        for kg in range(16):
            attn_block(st2, kg, no_alt=(kg < 2))
            if kg <= 5:
                kq_window(wk_sb, kT, kTa, 1, kg + 2)
            if kg == 1:
                attn_norm(st0)
            if kg in (6, 8, 10, 11, 12):
                drain_deferred(st1, 1)
            if kg == 9:
                load_weights_tail()
            if kg == 13:
                attn_norm(st1)
        st3 = attn_begin(3)
        for kg in range(16):
            attn_block(st3, kg)
            if kg == 1:
                attn_flush(st2)
            if kg == 3:
                attn_norm(st2)
        attn_flush(st3)
        attn_norm(st3, act_recip=True)

        # ---------------- output projection + residual ----------------
        for rw in range(4):
            rsl = slice(rw * 128, (rw + 1) * 128)
            pst = psB.tile([128, 512], F32, tag="pv", name="outp")
            ps = pst[:, 0:IN_DIM]
            for c in range(2):
                nc.tensor.matmul(
                    ps,
                    lhsT=catT[c][:, rsl],
                    rhs=wo_sb[c],
                    start=(c == 0),
                    stop=(c == 1),
                    skip_group_check=True,
                )
            o_sb = stream.tile([128, IN_DIM], F32, tag="o_sb", name="o_sb")
            nc.vector.tensor_add(out=o_sb, in0=ps, in1=xq_all[:, rw, :])
            nc.sync.dma_start(out=out[rsl, :], in_=o_sb)


_NC_CACHE = None


def _get_nc():
    global _NC_CACHE
    if _NC_CACHE is None:
        _NC_CACHE = build_nc()
    return _NC_CACHE


def kernel(_trace=False, **inputs):
    trace = _trace
    x = np.ascontiguousarray(np.asarray(inputs["x"], dtype=np.float32))
    coords = np.asarray(inputs["coords"], dtype=np.int32)
    coordsT = np.ascontiguousarray(coords.T)

    common = {
        "x": x,
        "coordsT": coordsT,
        "wq": np.ascontiguousarray(np.asarray(inputs["wq"], np.float32)),
        "bq": np.ascontiguousarray(np.asarray(inputs["bq"], np.float32)),
        "wk": np.ascontiguousarray(np.asarray(inputs["wk"], np.float32)),
        "bk": np.ascontiguousarray(np.asarray(inputs["bk"], np.float32)),
        "wv": np.ascontiguousarray(np.asarray(inputs["wv"], np.float32)),
        "bv": np.ascontiguousarray(np.asarray(inputs["bv"], np.float32)),
        "wo": np.ascontiguousarray(np.asarray(inputs["wo"], np.float32)),
        "bo": np.ascontiguousarray(np.asarray(inputs["bo"], np.float32)),
    }
    in_maps = []
    for c in range(NC):
        rsl = slice(c * R, (c + 1) * R)
        m = dict(common)
        m["xq"] = np.ascontiguousarray(x[rsl])
        m["cqT"] = np.ascontiguousarray(coordsT[:, rsl])
        in_maps.append(m)

    nc = _get_nc()
    res = run_bass_kernel_spmd(nc, in_maps, list(range(NC)), trace=trace)
    out = np.concatenate([res.results[c]["out"] for c in range(NC)], axis=0)
    if trace:
        return out, res
    return out


# revision 56
# speedup vs baseline: 1.0235x; 1.0235x over previous
"""Trainium2 Bass kernel for nn_MultiHeadAttention_3590592660317.

Sequence-sharded across 8 cores (512 rows each); each core redundantly
computes full K/V (cheap) plus its own Q rows, attention, and output
rows -> no collectives; the host concatenates the 8 output slices.

Schedule (v2): the ACT engine's 128 exps (~131us) are the floor; every
other engine is arranged around keeping ACT fed.
  - prep (~12us): weights, own-row + window-0 norms (ACT Square +
    Ln/Exp(-0.5) rsqrt, all in the natural_log_exp table set so the exp
    stream never reloads tables), qT (both groups), kT g0 w0, v 0-3.
  - build phase: pairs 0 AND 1 (both head-group 0) stream attention
    together over windows as they are built (8 exps/window), with
    window w+1's norm (gpsimd squares), scale+transpose (sync DGE), kT
    g0 and v chunks interleaved between blocks.  Pair 1's PV for
    kc>=22 is deferred (pts parked in SBUF) to offload the build-phase
    tensor engine.
  - phase 2: pairs 2,3 run back-to-back; kT g1 windows are built
    just-in-time under pair 2's exp stream, pair 1's deferred PV
    batches drain here, then pair flushes/norms and the output
    projection (bias pre-folded into the residual rows).
"""

import sys

if "/opt/trn_rl_repo" not in sys.path:
    sys.path.insert(0, "/opt/trn_rl_repo")

import numpy as np

import concourse.bass as bass
import concourse.tile as tile
from concourse import mybir
from concourse.bass_utils import run_bass_kernel_spmd
from concourse.tile import add_dep_helper

F32 = mybir.dt.float32
BF16 = mybir.dt.bfloat16
I32 = mybir.dt.int32

N = 4096
IN_DIM = 256
H = 8
HD = 32
DIN = 259
NC = 8
R = N // NC  # 512 rows per core

_KDIMS = (128, 128, 4)  # contraction chunks: 256 feats + (3 coords + ones)
DEFER_FROM = 11  # pair1 PV batches for pts-block kg >= this drain in phase 2


def _split_oversized_waits(nc, max_waits=1):
    """This walrus build only encodes one sync-wait per instruction; move
    excess on_wait conditions onto preceding same-engine NOPs."""
    nsplit = 0
    for f in nc.m.functions:
        for b in f.blocks:
            new_instrs = []
            for ins in b.instructions:
                si = getattr(ins, "sync_info", None)
                waits = list(si.on_wait) if si is not None and si.on_wait else []
                changed = False
                while len(waits) > max_waits:
                    chunk, waits = waits[:max_waits], waits[max_waits:]
                    nop = mybir.InstNoOp(name=f"{ins.name}-ws{nsplit}", ins=[], outs=[])
                    nop.engine = ins.engine
                    nop.sync_info = mybir.SyncInfo(on_wait=chunk, on_update=[])
                    new_instrs.append(nop)
                    nsplit += 1
                    changed = True
                if changed:
                    ins.sync_info = mybir.SyncInfo(
                        on_wait=waits, on_update=list(si.on_update or [])
                    )
                new_instrs.append(ins)
            b.instructions = new_instrs
    return nsplit


def build_nc(split=True):
    nc = bass.Bass()

    x = nc.dram_tensor("x", [N, IN_DIM], F32, kind="ExternalInput")
    coordsT = nc.dram_tensor("coordsT", [3, N], I32, kind="ExternalInput")
    xq = nc.dram_tensor("xq", [R, IN_DIM], F32, kind="ExternalInput")
    cqT = nc.dram_tensor("cqT", [3, R], I32, kind="ExternalInput")
    wq = nc.dram_tensor("wq", [H, DIN, HD], F32, kind="ExternalInput")
    bq = nc.dram_tensor("bq", [H, HD], F32, kind="ExternalInput")
    wk = nc.dram_tensor("wk", [H, DIN, HD], F32, kind="ExternalInput")
    bk = nc.dram_tensor("bk", [H, HD], F32, kind="ExternalInput")
    wv = nc.dram_tensor("wv", [H, DIN, HD], F32, kind="ExternalInput")
    bv = nc.dram_tensor("bv", [H, HD], F32, kind="ExternalInput")
    wo = nc.dram_tensor("wo", [IN_DIM, IN_DIM], F32, kind="ExternalInput")
    bo = nc.dram_tensor("bo", [IN_DIM], F32, kind="ExternalInput")
    out = nc.dram_tensor("out", [R, IN_DIM], F32, kind="ExternalOutput")

    with tile.TileContext(nc) as tc:
        _body(tc, nc, x, coordsT, xq, cqT, wq, bq, wk, bk, wv, bv, wo, bo, out)

    if split:
        _split_oversized_waits(nc)
    return nc


def _body(tc, nc, x, coordsT, xq, cqT, wq, bq, wk, bk, wv, bv, wo, bo, out):
    from contextlib import ExitStack

    ctx = ExitStack()
    with ctx:
        sing = ctx.enter_context(tc.tile_pool(name="sing", bufs=1))
        stream = ctx.enter_context(tc.tile_pool(name="stream", bufs=2))
        ftp = ctx.enter_context(tc.tile_pool(name="ftp", bufs=3))
        sqp = ctx.enter_context(tc.tile_pool(name="sqp", bufs=2))
        wstg = ctx.enter_context(tc.tile_pool(name="wstg", bufs=2))
        ptp = ctx.enter_context(tc.tile_pool(name="ptp", bufs=10))
        # deferred pair-1 pts live here (one-shot tiles, no rotation pressure)
        dfp = ctx.enter_context(tc.tile_pool(name="dfp", bufs=10))
        psA = ctx.enter_context(tc.tile_pool(name="psA", bufs=3, space="PSUM"))
        psB = ctx.enter_context(tc.tile_pool(name="psB", bufs=2, space="PSUM"))

        # ---------------- persistent SBUF tiles ----------------
        xcT0 = sing.tile([128, N], BF16, tag="xcT0", name="xcT0")
        xcT1 = sing.tile([128, N], BF16, tag="xcT1", name="xcT1")
        cnT = sing.tile([4, N], BF16, tag="cnT", name="cnT")  # coords + ones
        xcq0 = sing.tile([128, R], BF16, tag="xcq0", name="xcq0")
        xcq1 = sing.tile([128, R], BF16, tag="xcq1", name="xcq1")
        cnq = sing.tile([4, R], BF16, tag="cnq", name="cnq")
        kT = [sing.tile([128, N], BF16, tag=f"kT{g}", name=f"kT{g}") for g in range(2)]
        qT = [sing.tile([128, R], BF16, tag=f"qT{g}", name=f"qT{g}") for g in range(2)]
        # halves-swapped copies: head h%4 also lives at strip (h%4+2)%4
        kTa = [sing.tile([128, N], BF16, tag=f"kTa{g}", name=f"kTa{g}") for g in range(2)]
        qTa = [sing.tile([128, R], BF16, tag=f"qTa{g}", name=f"qTa{g}") for g in range(2)]
        # v natural layout with a ones column per head: [128, rc, 33*8]
        v_sb = sing.tile([128, N // 128, 33 * H], BF16, tag="vv", name="vv")
        catT = [sing.tile([128, R], BF16, tag=f"catT{g}", name=f"catT{g}") for g in range(2)]
        # x windows rotate through 4 slots (DMA -> squares -> scale spans 3)
        x_all = sing.tile([128, 4, 4, IN_DIM], F32, tag="x_all", name="x_all")
        xq_all = sing.tile([128, R // 128, IN_DIM], F32, tag="xq_all", name="xq_all")
        ss_all = sing.tile([128, 36], F32, tag="ss_all", name="ss_all")
        inv_all = sing.tile([128, 36], F32, tag="inv_all", name="inv_all")
        cq_f_g = sing.tile([4, R], F32, tag="cq_f_g", name="cq_f_g")

        # weights (bf16); chunk 2 is [3 coord rows + bias row]
        wk_sb = [sing.tile([_KDIMS[c], H, HD], BF16, tag=f"wk{c}", name=f"wk{c}") for c in range(3)]
        wq_sb = [sing.tile([_KDIMS[c], H, HD], BF16, tag=f"wq{c}", name=f"wq{c}") for c in range(3)]
        wv_sb = [sing.tile([_KDIMS[c], H * HD], BF16, tag=f"wv{c}", name=f"wv{c}") for c in range(3)]
        wo_sb = [sing.tile([128, IN_DIM], BF16, tag=f"wo{c}", name=f"wo{c}") for c in range(2)]
        bo_f32 = sing.tile([1, IN_DIM], F32, tag="bo32", name="bo32")
        ones_colf = sing.tile([1, 128], F32, tag="ones_colf", name="ones_colf")
        # block-select ones [64, 64]: row 0 -> out rows 0..31, row 32 -> out
        # rows 32..63 (engine partition offsets must be 32-aligned)
        ones2 = sing.tile([2 * HD, 2 * HD], F32, tag="ones2", name="ones2")
        bo_bc = sing.tile([128, IN_DIM], F32, tag="bo_bc", name="bo_bc")

        nc.vector.memset(ones_colf, 1.0)
        nc.vector.memset(ones2, 0.0)
        nc.vector.memset(ones2[0:1, 0:HD], 1.0)
        nc.vector.memset(ones2[HD:HD + 1, HD:2 * HD], 1.0)

        # ---------------- weight loads + casts ----------------
        def load_weights_main():
            # q/k/v projection weights + coords: everything the prep and
            # build phases need.  wo/bo load later (only the tail uses them).
            # coords + the c-chunk DMAs go first on their queues: the c=2
            # (coords/bias) matmul chunk gates every projection.
            # stage coords as [128,128]: rows 0-95 = 3 coord rows x 32 blocks,
            # rows 96-127 = 1.0 so the clamp+writeback also produces the ones
            # row of cnT (bias folding) without a wide memset
            nc.vector.memset(cq_f_g, 1.0)
            nc.gpsimd.dma_start(
                out=cq_f_g.bitcast(I32)[0:3], in_=cqT[:, :]
            )
            cwide = wstg.tile([128, 128], F32, tag="cwide", name="cwide")
            nc.vector.memset(cwide[96:128, :], 1.0)
            nc.sync.dma_start(
                out=cwide.bitcast(I32)[0:96],
                in_=coordsT[:, :].rearrange("c (j f) -> (c j) f", f=128),
            )
            stgs = {}
            for c in (2, 0, 1):
                kd = (128, 128, 3)[c]
                dsl = slice(c * 128, c * 128 + kd)
                for wi, (w_dram, b_dram) in enumerate(
                    ((wk, bk), (wq, bq), (wv, bv))
                ):
                    stg = sing.tile(
                        [128 if c != 2 else 4, H, HD], F32,
                        tag=f"stg{c}{wi}", name=f"stg{c}{wi}",
                    )
                    weng = (nc.gpsimd, nc.sync, nc.scalar)[c]
                    weng.dma_start(
                        out=stg[:kd], in_=w_dram[:, dsl, :].rearrange("h d k -> d h k")
                    )
                    if c == 2:
                        nc.gpsimd.dma_start(
                            out=stg[3:4],
                            in_=b_dram[:, :].rearrange("h k -> (h k)")[None, :]
                            .rearrange("a (h k) -> a h k", h=H),
                        )
                    stgs[(c, wi)] = stg
            # coord casts/clamps early on vector; c=2 weight casts (small,
            # gate the early projection chunk)
            nc.vector.tensor_copy(
                out=cwide[0:96], in_=cwide.bitcast(I32)[0:96]
            )
            cwb = wstg.tile([128, 128], BF16, tag="cwb", name="cwb")
            nc.vector.tensor_scalar_min(out=cwb, in0=cwide, scalar1=100.0)
            nc.sync.dma_start(out=cnT, in_=cwb)
            nc.vector.tensor_copy(out=cq_f_g[0:3], in_=cq_f_g.bitcast(I32)[0:3])
            nc.vector.tensor_scalar_min(out=cnq, in0=cq_f_g, scalar1=100.0)
            for wi, w_tile in enumerate((wk_sb[2], wq_sb[2], wv_sb[2])):
                w3 = (
                    w_tile if w_tile.shape[1] == H
                    else w_tile.rearrange("d (h k) -> d h k", h=H)
                )
                nc.vector.tensor_copy(out=w3[:4], in_=stgs[(2, wi)][:4])
            return stgs

        def load_weights_casts(stgs):
            # staging tiles are one-shot (distinct tags) so deferring these
            # casts is safe; emitted after the critical norm chain
            for c in (0, 1):
                for wi, w_tile in enumerate((wk_sb[c], wq_sb[c], wv_sb[c])):
                    w3 = (
                        w_tile if w_tile.shape[1] == H
                        else w_tile.rearrange("d (h k) -> d h k", h=H)
                    )
                    nc.vector.tensor_copy(out=w3[:128], in_=stgs[(c, wi)][:128])
            # ones columns of v (col 32 of each head's 33-wide block)
            v4 = v_sb.rearrange("p r (h c) -> p r h c", c=33)
            nc.vector.memset(v4[:, :, :, 32:33], 1.0)

        def load_weights_tail():
            for c in range(2):
                stg = wstg.tile([128, IN_DIM], F32, tag="wstg2", name="wstg2")
                nc.gpsimd.dma_start(out=stg, in_=wo[c * 128:(c + 1) * 128, :])
                nc.vector.tensor_copy(out=wo_sb[c], in_=stg)
            nc.gpsimd.dma_start(out=bo_f32, in_=bo[None, :])
            # broadcast bo once, then fold into the residual rows
            bct = psA.tile([128, 1024], F32, tag="att", name="bo_bc_ps")
            nc.tensor.matmul(
                bct[:, 0:IN_DIM], lhsT=ones_colf, rhs=bo_f32, start=True, stop=True
            )
            nc.vector.tensor_copy(out=bo_bc, in_=bct[:, 0:IN_DIM])
            for rw in range(4):
                nc.vector.tensor_add(
                    out=xq_all[:, rw, :], in0=xq_all[:, rw, :], in1=bo_bc
                )

        # ---------------- norm machinery ----------------
        def dma_x_window(w, eng):
            eng.dma_start(
                out=x_all[:, w % 4],
                in_=x[w * 512:(w + 1) * 512, :].rearrange("(c p) d -> p c d", p=128),
            )

        def squares_act(xbuf, nchunk, ss_col):
            # ACT Square with accumulate: used in prep only (table co-resident
            # with Exp/Ln so the exp stream never reloads)
            for i in range(nchunk):
                sq = stream.tile([128, IN_DIM], F32, tag="sqa", name="sqa")
                nc.scalar.activation(
                    out=sq, in_=xbuf[:, i, :],
                    func=mybir.ActivationFunctionType.Square,
                    accum_out=ss_all[:, ss_col + i:ss_col + i + 1],
                )

        def squares_gp(w):
            # in-stream norms go to gpsimd so ACT only runs exps
            sq = sqp.tile([128, 4, IN_DIM], F32, tag="sqg", name="sqg")
            nc.gpsimd.tensor_mul(out=sq, in0=x_all[:, w % 4], in1=x_all[:, w % 4])
            nc.vector.tensor_reduce(
                out=ss_all[:, 4 * w:4 * w + 4], in_=sq,
                axis=mybir.AxisListType.X, op=mybir.AluOpType.add,
            )

        def inv_lnexp(col0, ncols):
            # 1/sqrt(ss) = exp(-0.5*ln(ss)); Ln and Exp share a table set
            lnb = stream.tile([128, 4], F32, tag="lnb", name="lnb")
            nc.scalar.activation(
                out=lnb, in_=ss_all[:, col0:col0 + ncols],
                func=mybir.ActivationFunctionType.Ln,
            )
            nc.scalar.activation(
                out=inv_all[:, col0:col0 + ncols], in_=lnb,
                func=mybir.ActivationFunctionType.Exp, scale=-0.5,
            )

        def scale_mul(rc, lo, hi, xap, ss_col):
            # split the scaled chunk into window-combined lo/hi tiles so one
            # batched xbar transpose per half covers the whole 512-row window
            inv1 = inv_all[:, ss_col:ss_col + 1]
            nc.vector.tensor_scalar_mul(
                out=lo[:, rc % 4, :], in0=xap[:, 0:128], scalar1=inv1
            )
            nc.vector.tensor_scalar_mul(
                out=hi[:, rc % 4, :], in0=xap[:, 128:256], scalar1=inv1
            )

        def scale_transpose(nw, t0, t1, lo, hi, eng0=None, eng1=None):
            nsl = slice(nw * 512, (nw + 1) * 512)
            (eng0 or nc.sync).dma_start_transpose(
                out=t0[:, nsl].rearrange("f (g r) -> f g r", r=128),
                in_=lo.rearrange("p g r -> p (g r)"),
            )
            (eng1 or nc.sync).dma_start_transpose(
                out=t1[:, nsl].rearrange("f (g r) -> f g r", r=128),
                in_=hi.rearrange("p g r -> p (g r)"),
            )

        def xcT_chunk(c, full=True):
            if full:
                return (xcT0, xcT1, cnT)[c]
            return (xcq0, xcq1, cnq)[c]

        # ---------------- projections ----------------
        def kq_window(w_sb_, dst, alt, g, nw, full=True, alt_eng=None):
            nsl = slice(nw * 512, (nw + 1) * 512)
            pst = psA.tile([128, 1024], F32, tag="att", name="proj")
            ps = pst[:, 0:512]
            # c=2 (coords+bias) first: its operands are ready earliest
            for c in (2, 0, 1):
                kd = _KDIMS[c]
                nc.tensor.matmul(
                    ps,
                    lhsT=w_sb_[c][:kd, g * 4:g * 4 + 4, :].rearrange(
                        "d h k -> d (h k)"
                    ),
                    rhs=xcT_chunk(c, full)[:kd, nsl],
                    start=(c == 2),
                    stop=(c == 1),
                    skip_group_check=True,
                )
            nc.vector.tensor_copy(out=dst[g][:, nsl], in_=ps)
            if alt is not None:
                ae = alt_eng or nc.vector
                ae.tensor_copy(out=alt[g][0:64, nsl], in_=ps[64:128, :])
                ae.tensor_copy(out=alt[g][64:128, nsl], in_=ps[0:64, :])

        def v_chunk(rc):
            rsl = slice(rc * 128, (rc + 1) * 128)
            pst = psA.tile([128, 1024], F32, tag="att", name="projv")
            ps = pst[:, 0:H * HD]
            for c in (2, 0, 1):
                kd = _KDIMS[c]
                nc.tensor.matmul(
                    ps,
                    lhsT=xcT_chunk(c)[:kd, rsl],
                    rhs=wv_sb[c][:kd, :],
                    start=(c == 2),
                    stop=(c == 1),
                    skip_group_check=True,
                )
            nc.vector.tensor_copy(
                out=v_sb[:, rc, :].rearrange("p (h k) -> p h k", h=H)[:, :, 0:HD],
                in_=ps.rearrange("p (h k) -> p h k", h=H),
            )

        # ---------------- attention ----------------
        SCALE = 1.0 / float(np.sqrt(HD))

        class _St:
            pass

        def attn_begin(pair, defer_from=None):
            st = _St()
            st.pair = pair
            st.g = pair // 2
            st.hp0 = (pair % 2) * 2
            pvt = psB.tile([128, 512], F32, tag="pv", name="pv")
            st.pvps = pvt[0:97, :]
            st.prev_pts = None
            st.prev_kg = None
            st.pv_last = None
            st.defer_from = defer_from
            st.deferred = []
            return st

        def _pv_batch(st, kgp, pts, qk_last=None):
            for sub in range(2):
                kc = 2 * kgp + sub
                for a in range(2):
                    h = 2 * st.pair + a
                    mm = nc.tensor.matmul(
                        st.pvps[64 * a:64 * a + 33, :],
                        lhsT=v_sb[:, kc, 33 * h:33 * h + 33],
                        rhs=pts[a][:, sub * 512:(sub + 1) * 512],
                        start=(kc == 0),
                        stop=(kc == N // 128 - 1),
                        tile_position=(0, 64 * a),
                        skip_group_check=True,
                    )
                    if qk_last is not None:
                        add_dep_helper(mm.ins, qk_last.ins, sync=False)
                    st.pv_last = mm

        def attn_block(st, kg, no_alt=False):
            g, hp0 = st.g, st.hp0
            qk_last = None
            pts = None
            if kg < 16:
                attps = [
                    psA.tile([128, 1024], F32, tag="att", name="att")
                    for _ in range(2)
                ]
                for sub in range(2):
                    kc = 2 * kg + sub
                    ksl = slice(kc * 128, (kc + 1) * 128)
                    par = (kc % 2) if not no_alt else 0
                    kt_src = (kT, kTa)[par]
                    qt_src = (qT, qTa)[par]
                    for a in range(2):
                        s = (hp0 + a + 2 * par) % 4
                        mm = nc.tensor.matmul(
                            attps[a][:, sub * 512:(sub + 1) * 512],
                            lhsT=kt_src[g][32 * s:32 * s + 32, ksl],
                            rhs=qt_src[g][32 * s:32 * s + 32, :],
                            start=True,
                            stop=True,
                            tile_position=(32 * s, 0),
                        )
                        if st.pv_last is not None:
                            add_dep_helper(mm.ins, st.pv_last.ins, sync=False)
                        qk_last = mm
                pts = []
                will_defer = st.defer_from is not None and kg >= st.defer_from
                pool = dfp if will_defer else ptp
                for a in range(2):
                    pt = pool.tile([128, 1024], BF16, tag="pt", name="pt")
                    nc.scalar.activation(
                        out=pt,
                        in_=attps[a],
                        func=mybir.ActivationFunctionType.Exp,
                        scale=SCALE,
                    )
                    pts.append(pt)
            if st.prev_pts is not None:
                if st.defer_from is not None and st.prev_kg >= st.defer_from:
                    st.deferred.append((st.prev_kg, st.prev_pts))
                else:
                    _pv_batch(st, st.prev_kg, st.prev_pts, qk_last)
            st.prev_pts = pts
            st.prev_kg = kg if kg < 16 else None

        def attn_flush(st):
            attn_block(st, 16)

        def drain_deferred(st, nbatch=1):
            for _ in range(nbatch):
                if st.deferred:
                    kgp, pts = st.deferred.pop(0)
                    _pv_batch(st, kgp, pts)

        def attn_norm(st, act_recip=False):
            g, hp0 = st.g, st.hp0
            # sums land at rows 0 and 32 (32-aligned); rows in between hold
            # 1.0 so reciprocal stays finite (they hit zero lhsT weights)
            s_sb = stream.tile([2 * HD, 512], F32, tag="s_sb", name="s_sb")
            nc.gpsimd.memset(s_sb, 1.0)
            for a in range(2):
                nc.vector.tensor_scalar_add(
                    out=s_sb[HD * a:HD * a + 1, :],
                    in0=st.pvps[64 * a + 32:64 * a + 33, :],
                    scalar1=1e-6,
                )
            r_sb = stream.tile([2 * HD, 512], F32, tag="r_sb", name="r_sb")
            if act_recip:
                # tail only (ACT idle there): 1/s = exp(-ln(s)), same table
                # set as the exp stream
                lnr = stream.tile([2 * HD, 512], F32, tag="lnr", name="lnr")
                nc.scalar.activation(
                    out=lnr, in_=s_sb, func=mybir.ActivationFunctionType.Ln
                )
                nc.scalar.activation(
                    out=r_sb, in_=lnr,
                    func=mybir.ActivationFunctionType.Exp, scale=-1.0,
                )
            else:
                nc.vector.reciprocal(out=r_sb, in_=s_sb)
            rbct = psA.tile([128, 1024], F32, tag="att", name="rbc")
            rbc = rbct[0:2 * HD, 0:512]
            nc.tensor.matmul(rbc, lhsT=ones2, rhs=r_sb, start=True, stop=True)
            rbc_sb = stream.tile([2 * HD, 512], F32, tag="rbc_sb", name="rbc_sb")
            nc.vector.tensor_copy(out=rbc_sb, in_=rbc)
            for a in range(2):
                nc.vector.tensor_mul(
                    out=catT[g][32 * (hp0 + a):32 * (hp0 + a) + 32, :],
                    in0=st.pvps[64 * a:64 * a + 32, :],
                    in1=rbc_sb[32 * a:32 * a + 32, :],
                )

        # ---------------- emission ----------------
        # critical x DMAs chunked + first so norms start as rows land
        for rc in range(4):
            eng = (nc.sync, nc.scalar)[rc % 2]
            eng.dma_start(
                out=xq_all[:, rc, :], in_=xq[rc * 128:(rc + 1) * 128, :]
            )
        for rc in range(4):
            eng = (nc.sync, nc.scalar, nc.gpsimd)[rc % 3]
            eng.dma_start(
                out=x_all[:, 0, rc, :], in_=x[rc * 128:(rc + 1) * 128, :]
            )
        stgs = load_weights_main()

        squares_act(xq_all, 4, 32)
        inv_lnexp(32, 4)
        squares_act(x_all[:, 0], 4, 0)
        inv_lnexp(0, 4)

        flo = ftp.tile([128, 4, 128], BF16, tag="flo", name="flo")
        fhi = ftp.tile([128, 4, 128], BF16, tag="fhi", name="fhi")
        for rc in range(4):
            scale_mul(rc, flo, fhi, xq_all[:, rc, :], 32 + rc)
        scale_transpose(0, xcq0, xcq1, flo, fhi, eng0=nc.sync, eng1=nc.sync)
        flo = ftp.tile([128, 4, 128], BF16, tag="flo", name="flo")
        fhi = ftp.tile([128, 4, 128], BF16, tag="fhi", name="fhi")
        for rc in range(4):
            scale_mul(rc, flo, fhi, x_all[:, 0, rc, :], rc)
        scale_transpose(0, xcT0, xcT1, flo, fhi, eng0=nc.scalar, eng1=nc.sync)
        load_weights_casts(stgs)
        for g in range(2):
            kq_window(wq_sb, qT, qTa, g, 0, full=False)
        dma_x_window(1, nc.gpsimd)
        dma_x_window(2, nc.gpsimd)
        kq_window(wk_sb, kT, kTa, 0, 0)
        for rc in range(4):
            v_chunk(rc)
        squares_gp(1)

        # -------- build phase: pairs 0+1 stream over windows as built -----
        st0 = attn_begin(0)
        st1 = attn_begin(1, defer_from=DEFER_FROM)
        for nw in range(8):
            wn = nw + 1
            if wn + 2 <= 7:
                dma_x_window(wn + 2, nc.gpsimd)
            if wn + 1 <= 7:
                squares_gp(wn + 1)
            if wn <= 7:
                inv_lnexp(4 * wn, 4)
                flo = ftp.tile([128, 4, 128], BF16, tag="flo", name="flo")
                fhi = ftp.tile([128, 4, 128], BF16, tag="fhi", name="fhi")
                scale_mul(0, flo, fhi, x_all[:, wn % 4, 0, :], 4 * wn + 0)
                scale_mul(1, flo, fhi, x_all[:, wn % 4, 1, :], 4 * wn + 1)
            attn_block(st0, 2 * nw)
            if wn <= 7:
                scale_mul(2, flo, fhi, x_all[:, wn % 4, 2, :], 4 * wn + 2)
                scale_mul(3, flo, fhi, x_all[:, wn % 4, 3, :], 4 * wn + 3)
                scale_transpose(wn, xcT0, xcT1, flo, fhi)
            attn_block(st1, 2 * nw)
            if wn <= 7:
                kq_window(wk_sb, kT, kTa, 0, wn)
            elif nw == 7:
                kq_window(wk_sb, kT, kTa, 1, 0)
            attn_block(st0, 2 * nw + 1)
            if wn <= 7:
                v_chunk(4 * wn + 0)
                v_chunk(4 * wn + 1)
            elif nw == 7:
                kq_window(wk_sb, kT, kTa, 1, 1)
            attn_block(st1, 2 * nw + 1)
            if wn <= 7:
                v_chunk(4 * wn + 2)
                v_chunk(4 * wn + 3)

        attn_flush(st0)
        attn_flush(st1)  # lands in the deferred list

        # -------- phase 2: pairs 2,3; kT g1 + deferred PV drain in-stream --
        st2 = attn_begin(2)
        for kg in range(16):
            attn_block(st2, kg, no_alt=(kg < 2))
            if kg <= 5:
                kq_window(wk_sb, kT, kTa, 1, kg + 2)
            if kg == 1:
                attn_norm(st0)
            if kg in (6, 8, 10, 11, 12):
                drain_deferred(st1, 1)
            if kg == 9:
                load_weights_tail()
            if kg == 13:
                attn_norm(st1)
        st3 = attn_begin(3)
        for kg in range(16):
            attn_block(st3, kg)
            if kg == 1:
                attn_flush(st2)
            if kg == 3:
                attn_norm(st2)
        attn_flush(st3)
        attn_norm(st3, act_recip=True)

        # ---------------- output projection + residual ----------------
        for rw in range(4):
            rsl = slice(rw * 128, (rw + 1) * 128)
            pst = psB.tile([128, 512], F32, tag="pv", name="outp")
            ps = pst[:, 0:IN_DIM]
            for c in range(2):
                nc.tensor.matmul(
                    ps,
                    lhsT=catT[c][:, rsl],
                    rhs=wo_sb[c],
                    start=(c == 0),
                    stop=(c == 1),
                    skip_group_check=True,
                )
            o_sb = stream.tile([128, IN_DIM], F32, tag="o_sb", name="o_sb")
            nc.vector.tensor_add(out=o_sb, in0=ps, in1=xq_all[:, rw, :])
            nc.sync.dma_start(out=out[rsl, :], in_=o_sb)


_NC_CACHE = None


def _get_nc():
    global _NC_CACHE
    if _NC_CACHE is None:
        _NC_CACHE = build_nc()
    return _NC_CACHE


def kernel(_trace=False, **inputs):
    trace = _trace
    x = np.ascontiguousarray(np.asarray(inputs["x"], dtype=np.float32))
    coords = np.asarray(inputs["coords"], dtype=np.int32)
    coordsT = np.ascontiguousarray(coords.T)

    common = {
        "x": x,
        "coordsT": coordsT,
        "wq": np.ascontiguousarray(np.asarray(inputs["wq"], np.float32)),
        "bq": np.ascontiguousarray(np.asarray(inputs["bq"], np.float32)),
        "wk": np.ascontiguousarray(np.asarray(inputs["wk"], np.float32)),
        "bk": np.ascontiguousarray(np.asarray(inputs["bk"], np.float32)),
        "wv": np.ascontiguousarray(np.asarray(inputs["wv"], np.float32)),
        "bv": np.ascontiguousarray(np.asarray(inputs["bv"], np.float32)),
        "wo": np.ascontiguousarray(np.asarray(inputs["wo"], np.float32)),
        "bo": np.ascontiguousarray(np.asarray(inputs["bo"], np.float32)),
    }
    in_maps = []
    for c in range(NC):
        rsl = slice(c * R, (c + 1) * R)
        m = dict(common)
        m["xq"] = np.ascontiguousarray(x[rsl])
        m["cqT"] = np.ascontiguousarray(coordsT[:, rsl])
        in_maps.append(m)

    nc = _get_nc()
    res = run_bass_kernel_spmd(nc, in_maps, list(range(NC)), trace=trace)
    out = np.concatenate([res.results[c]["out"] for c in range(NC)], axis=0)
    if trace:
        return out, res
    return out


# revision 57
# speedup vs baseline: 1.0683x; 1.0438x over previous
"""Trainium2 Bass kernel for nn_MultiHeadAttention_3590592660317.

Sequence-sharded across 8 cores (512 rows each); each core redundantly
computes full K/V (cheap) plus its own Q rows, attention, and output
rows -> no collectives; the host concatenates the 8 output slices.

Schedule (v2): the ACT engine's 128 exps (~131us) are the floor; every
other engine is arranged around keeping ACT fed.
  - prep (~12us): weights, own-row + window-0 norms (ACT Square +
    Ln/Exp(-0.5) rsqrt, all in the natural_log_exp table set so the exp
    stream never reloads tables), qT (both groups), kT g0 w0, v 0-3.
  - build phase: pairs 0 AND 1 (both head-group 0) stream attention
    together over windows as they are built (8 exps/window), with
    window w+1's norm (gpsimd squares), scale+transpose (sync DGE), kT
    g0 and v chunks interleaved between blocks.  Pair 1's PV for
    kc>=22 is deferred (pts parked in SBUF) to offload the build-phase
    tensor engine.
  - phase 2: pairs 2,3 run back-to-back; kT g1 windows are built
    just-in-time under pair 2's exp stream, pair 1's deferred PV
    batches drain here, then pair flushes/norms and the output
    projection (bias pre-folded into the residual rows).
"""

import sys

if "/opt/trn_rl_repo" not in sys.path:
    sys.path.insert(0, "/opt/trn_rl_repo")

import numpy as np

import concourse.bass as bass
import concourse.tile as tile
from concourse import mybir
from concourse.bass_utils import run_bass_kernel_spmd
from concourse.tile import add_dep_helper

F32 = mybir.dt.float32
BF16 = mybir.dt.bfloat16
I32 = mybir.dt.int32

N = 4096
IN_DIM = 256
H = 8
HD = 32
DIN = 259
NC = 8
R = N // NC  # 512 rows per core

_KDIMS = (128, 128, 4)  # contraction chunks: 256 feats + (3 coords + ones)
DEFER_FROM = 11  # pair1 PV batches for pts-block kg >= this drain in phase 2


def _split_oversized_waits(nc, max_waits=1):
    """This walrus build only encodes one sync-wait per instruction; move
    excess on_wait conditions onto preceding same-engine NOPs."""
    nsplit = 0
    for f in nc.m.functions:
        for b in f.blocks:
            new_instrs = []
            for ins in b.instructions:
                si = getattr(ins, "sync_info", None)
                waits = list(si.on_wait) if si is not None and si.on_wait else []
                changed = False
                while len(waits) > max_waits:
                    chunk, waits = waits[:max_waits], waits[max_waits:]
                    nop = mybir.InstNoOp(name=f"{ins.name}-ws{nsplit}", ins=[], outs=[])
                    nop.engine = ins.engine
                    nop.sync_info = mybir.SyncInfo(on_wait=chunk, on_update=[])
                    new_instrs.append(nop)
                    nsplit += 1
                    changed = True
                if changed:
                    ins.sync_info = mybir.SyncInfo(
                        on_wait=waits, on_update=list(si.on_update or [])
                    )
                new_instrs.append(ins)
            b.instructions = new_instrs
    return nsplit


def build_nc(split=True):
    nc = bass.Bass()

    x = nc.dram_tensor("x", [N, IN_DIM], F32, kind="ExternalInput")
    coordsT = nc.dram_tensor("coordsT", [3, N], I32, kind="ExternalInput")
    xq = nc.dram_tensor("xq", [R, IN_DIM], F32, kind="ExternalInput")
    cqT = nc.dram_tensor("cqT", [3, R], I32, kind="ExternalInput")
    wq = nc.dram_tensor("wq", [H, DIN, HD], F32, kind="ExternalInput")
    bq = nc.dram_tensor("bq", [H, HD], F32, kind="ExternalInput")
    wk = nc.dram_tensor("wk", [H, DIN, HD], F32, kind="ExternalInput")
    bk = nc.dram_tensor("bk", [H, HD], F32, kind="ExternalInput")
    wv = nc.dram_tensor("wv", [H, DIN, HD], F32, kind="ExternalInput")
    bv = nc.dram_tensor("bv", [H, HD], F32, kind="ExternalInput")
    wo = nc.dram_tensor("wo", [IN_DIM, IN_DIM], F32, kind="ExternalInput")
    bo = nc.dram_tensor("bo", [IN_DIM], F32, kind="ExternalInput")
    out = nc.dram_tensor("out", [R, IN_DIM], F32, kind="ExternalOutput")

    with tile.TileContext(nc) as tc:
        _body(tc, nc, x, coordsT, xq, cqT, wq, bq, wk, bk, wv, bv, wo, bo, out)

    if split:
        _split_oversized_waits(nc)
    return nc


def _body(tc, nc, x, coordsT, xq, cqT, wq, bq, wk, bk, wv, bv, wo, bo, out):
    from contextlib import ExitStack

    ctx = ExitStack()
    with ctx:
        sing = ctx.enter_context(tc.tile_pool(name="sing", bufs=1))
        stream = ctx.enter_context(tc.tile_pool(name="stream", bufs=2))
        ftp = ctx.enter_context(tc.tile_pool(name="ftp", bufs=3))
        sqp = ctx.enter_context(tc.tile_pool(name="sqp", bufs=2))
        wstg = ctx.enter_context(tc.tile_pool(name="wstg", bufs=2))
        ptp = ctx.enter_context(tc.tile_pool(name="ptp", bufs=10))
        # deferred pair-1 pts live here (one-shot tiles, no rotation pressure)
        dfp = ctx.enter_context(tc.tile_pool(name="dfp", bufs=10))
        psA = ctx.enter_context(tc.tile_pool(name="psA", bufs=3, space="PSUM"))
        psB = ctx.enter_context(tc.tile_pool(name="psB", bufs=2, space="PSUM"))

        # ---------------- persistent SBUF tiles ----------------
        xcT0 = sing.tile([128, N], BF16, tag="xcT0", name="xcT0")
        xcT1 = sing.tile([128, N], BF16, tag="xcT1", name="xcT1")
        cnT = sing.tile([4, N], BF16, tag="cnT", name="cnT")  # coords + ones
        xcq0 = sing.tile([128, R], BF16, tag="xcq0", name="xcq0")
        xcq1 = sing.tile([128, R], BF16, tag="xcq1", name="xcq1")
        cnq = sing.tile([4, R], BF16, tag="cnq", name="cnq")
        kT = [sing.tile([128, N], BF16, tag=f"kT{g}", name=f"kT{g}") for g in range(2)]
        qT = [sing.tile([128, R], BF16, tag=f"qT{g}", name=f"qT{g}") for g in range(2)]
        # halves-swapped copies: head h%4 also lives at strip (h%4+2)%4
        kTa = [sing.tile([128, N], BF16, tag=f"kTa{g}", name=f"kTa{g}") for g in range(2)]
        qTa = [sing.tile([128, R], BF16, tag=f"qTa{g}", name=f"qTa{g}") for g in range(2)]
        # v natural layout with a ones column per head: [128, rc, 33*8]
        v_sb = sing.tile([128, N // 128, 33 * H], BF16, tag="vv", name="vv")
        catT = [sing.tile([128, R], BF16, tag=f"catT{g}", name=f"catT{g}") for g in range(2)]
        # x windows rotate through 4 slots (DMA -> squares -> scale spans 3)
        x_all = sing.tile([128, 4, 4, IN_DIM], F32, tag="x_all", name="x_all")
        xq_all = sing.tile([128, R // 128, IN_DIM], F32, tag="xq_all", name="xq_all")
        ss_all = sing.tile([128, 36], F32, tag="ss_all", name="ss_all")
        inv_all = sing.tile([128, 36], F32, tag="inv_all", name="inv_all")
        cq_f_g = sing.tile([4, R], F32, tag="cq_f_g", name="cq_f_g")

        # weights (bf16); chunk 2 is [3 coord rows + bias row]
        wk_sb = [sing.tile([_KDIMS[c], H, HD], BF16, tag=f"wk{c}", name=f"wk{c}") for c in range(3)]
        wq_sb = [sing.tile([_KDIMS[c], H, HD], BF16, tag=f"wq{c}", name=f"wq{c}") for c in range(3)]
        wv_sb = [sing.tile([_KDIMS[c], H * HD], BF16, tag=f"wv{c}", name=f"wv{c}") for c in range(3)]
        wo_sb = [sing.tile([128, IN_DIM], BF16, tag=f"wo{c}", name=f"wo{c}") for c in range(2)]
        bo_f32 = sing.tile([1, IN_DIM], F32, tag="bo32", name="bo32")
        ones_colf = sing.tile([1, 128], F32, tag="ones_colf", name="ones_colf")
        # block-select ones [64, 64]: row 0 -> out rows 0..31, row 32 -> out
        # rows 32..63 (engine partition offsets must be 32-aligned)
        ones2 = sing.tile([2 * HD, 2 * HD], F32, tag="ones2", name="ones2")
        bo_bc = sing.tile([128, IN_DIM], F32, tag="bo_bc", name="bo_bc")

        nc.vector.memset(ones_colf, 1.0)
        nc.vector.memset(ones2, 0.0)
        nc.vector.memset(ones2[0:1, 0:HD], 1.0)
        nc.vector.memset(ones2[HD:HD + 1, HD:2 * HD], 1.0)

        # ---------------- weight loads + casts ----------------
        def load_weights_main():
            # q/k/v projection weights + coords: everything the prep and
            # build phases need.  wo/bo load later (only the tail uses them).
            # coords + the c-chunk DMAs go first on their queues: the c=2
            # (coords/bias) matmul chunk gates every projection.
            # stage coords as [128,128]: rows 0-95 = 3 coord rows x 32 blocks,
            # rows 96-127 = 1.0 so the clamp+writeback also produces the ones
            # row of cnT (bias folding) without a wide memset
            nc.vector.memset(cq_f_g, 1.0)
            nc.gpsimd.dma_start(
                out=cq_f_g.bitcast(I32)[0:3], in_=cqT[:, :]
            )
            cwide = wstg.tile([128, 128], F32, tag="cwide", name="cwide")
            nc.vector.memset(cwide[96:128, :], 1.0)
            nc.sync.dma_start(
                out=cwide.bitcast(I32)[0:96],
                in_=coordsT[:, :].rearrange("c (j f) -> (c j) f", f=128),
            )
            stgs = {}
            for c in (2, 0, 1):
                kd = (128, 128, 3)[c]
                dsl = slice(c * 128, c * 128 + kd)
                for wi, (w_dram, b_dram) in enumerate(
                    ((wk, bk), (wq, bq), (wv, bv))
                ):
                    stg = sing.tile(
                        [128 if c != 2 else 4, H, HD], F32,
                        tag=f"stg{c}{wi}", name=f"stg{c}{wi}",
                    )
                    weng = (nc.gpsimd, nc.sync, nc.scalar)[c]
                    weng.dma_start(
                        out=stg[:kd], in_=w_dram[:, dsl, :].rearrange("h d k -> d h k")
                    )
                    if c == 2:
                        nc.gpsimd.dma_start(
                            out=stg[3:4],
                            in_=b_dram[:, :].rearrange("h k -> (h k)")[None, :]
                            .rearrange("a (h k) -> a h k", h=H),
                        )
                    stgs[(c, wi)] = stg
            # coord casts/clamps early on vector; c=2 weight casts (small,
            # gate the early projection chunk)
            nc.vector.tensor_copy(
                out=cwide[0:96], in_=cwide.bitcast(I32)[0:96]
            )
            cwb = wstg.tile([128, 128], BF16, tag="cwb", name="cwb")
            nc.vector.tensor_scalar_min(out=cwb, in0=cwide, scalar1=100.0)
            nc.sync.dma_start(out=cnT, in_=cwb)
            nc.vector.tensor_copy(out=cq_f_g[0:3], in_=cq_f_g.bitcast(I32)[0:3])
            nc.vector.tensor_scalar_min(out=cnq, in0=cq_f_g, scalar1=100.0)
            for wi, w_tile in enumerate((wk_sb[2], wq_sb[2], wv_sb[2])):
                w3 = (
                    w_tile if w_tile.shape[1] == H
                    else w_tile.rearrange("d (h k) -> d h k", h=H)
                )
                nc.vector.tensor_copy(out=w3[:4], in_=stgs[(2, wi)][:4])
            return stgs

        def load_weights_casts(stgs):
            # staging tiles are one-shot (distinct tags) so deferring these
            # casts is safe; emitted after the critical norm chain
            for c in (0, 1):
                for wi, w_tile in enumerate((wk_sb[c], wq_sb[c], wv_sb[c])):
                    w3 = (
                        w_tile if w_tile.shape[1] == H
                        else w_tile.rearrange("d (h k) -> d h k", h=H)
                    )
                    nc.vector.tensor_copy(out=w3[:128], in_=stgs[(c, wi)][:128])
            # ones columns of v (col 32 of each head's 33-wide block)
            v4 = v_sb.rearrange("p r (h c) -> p r h c", c=33)
            nc.vector.memset(v4[:, :, :, 32:33], 1.0)

        def load_weights_tail():
            for c in range(2):
                stg = wstg.tile([128, IN_DIM], F32, tag="wstg2", name="wstg2")
                nc.gpsimd.dma_start(out=stg, in_=wo[c * 128:(c + 1) * 128, :])
                nc.vector.tensor_copy(out=wo_sb[c], in_=stg)
            nc.gpsimd.dma_start(out=bo_f32, in_=bo[None, :])
            # broadcast bo once, then fold into the residual rows
            bct = psA.tile([128, 1024], F32, tag="att", name="bo_bc_ps")
            nc.tensor.matmul(
                bct[:, 0:IN_DIM], lhsT=ones_colf, rhs=bo_f32, start=True, stop=True
            )
            nc.vector.tensor_copy(out=bo_bc, in_=bct[:, 0:IN_DIM])
            for rw in range(4):
                nc.vector.tensor_add(
                    out=xq_all[:, rw, :], in0=xq_all[:, rw, :], in1=bo_bc
                )

        # ---------------- norm machinery ----------------
        def dma_x_window(w, eng):
            eng.dma_start(
                out=x_all[:, w % 4],
                in_=x[w * 512:(w + 1) * 512, :].rearrange("(c p) d -> p c d", p=128),
            )

        def squares_act(xbuf, nchunk, ss_col):
            # ACT Square with accumulate: used in prep only (table co-resident
            # with Exp/Ln so the exp stream never reloads)
            for i in range(nchunk):
                sq = stream.tile([128, IN_DIM], F32, tag="sqa", name="sqa")
                nc.scalar.activation(
                    out=sq, in_=xbuf[:, i, :],
                    func=mybir.ActivationFunctionType.Square,
                    accum_out=ss_all[:, ss_col + i:ss_col + i + 1],
                )

        def squares_gp(w):
            # in-stream norms go to gpsimd so ACT only runs exps
            sq = sqp.tile([128, 4, IN_DIM], F32, tag="sqg", name="sqg")
            nc.gpsimd.tensor_mul(out=sq, in0=x_all[:, w % 4], in1=x_all[:, w % 4])
            nc.vector.tensor_reduce(
                out=ss_all[:, 4 * w:4 * w + 4], in_=sq,
                axis=mybir.AxisListType.X, op=mybir.AluOpType.add,
            )

        def inv_lnexp(col0, ncols):
            # 1/sqrt(ss) = exp(-0.5*ln(ss)); Ln and Exp share a table set
            lnb = stream.tile([128, 4], F32, tag="lnb", name="lnb")
            nc.scalar.activation(
                out=lnb, in_=ss_all[:, col0:col0 + ncols],
                func=mybir.ActivationFunctionType.Ln,
            )
            nc.scalar.activation(
                out=inv_all[:, col0:col0 + ncols], in_=lnb,
                func=mybir.ActivationFunctionType.Exp, scale=-0.5,
            )

        def scale_mul(rc, lo, hi, xap, ss_col):
            # split the scaled chunk into window-combined lo/hi tiles so one
            # batched xbar transpose per half covers the whole 512-row window
            inv1 = inv_all[:, ss_col:ss_col + 1]
            nc.vector.tensor_scalar_mul(
                out=lo[:, rc % 4, :], in0=xap[:, 0:128], scalar1=inv1
            )
            nc.vector.tensor_scalar_mul(
                out=hi[:, rc % 4, :], in0=xap[:, 128:256], scalar1=inv1
            )

        def scale_transpose(nw, t0, t1, lo, hi, eng0=None, eng1=None):
            nsl = slice(nw * 512, (nw + 1) * 512)
            (eng0 or nc.sync).dma_start_transpose(
                out=t0[:, nsl].rearrange("f (g r) -> f g r", r=128),
                in_=lo.rearrange("p g r -> p (g r)"),
            )
            (eng1 or nc.sync).dma_start_transpose(
                out=t1[:, nsl].rearrange("f (g r) -> f g r", r=128),
                in_=hi.rearrange("p g r -> p (g r)"),
            )

        def xcT_chunk(c, full=True):
            if full:
                return (xcT0, xcT1, cnT)[c]
            return (xcq0, xcq1, cnq)[c]

        # ---------------- projections ----------------
        def kq_window(w_sb_, dst, alt, g, nw, full=True, alt_eng=None):
            nsl = slice(nw * 512, (nw + 1) * 512)
            pst = psA.tile([128, 1024], F32, tag="att", name="proj")
            ps = pst[:, 0:512]
            # c=2 (coords+bias) first: its operands are ready earliest
            for c in (2, 0, 1):
                kd = _KDIMS[c]
                nc.tensor.matmul(
                    ps,
                    lhsT=w_sb_[c][:kd, g * 4:g * 4 + 4, :].rearrange(
                        "d h k -> d (h k)"
                    ),
                    rhs=xcT_chunk(c, full)[:kd, nsl],
                    start=(c == 2),
                    stop=(c == 1),
                    skip_group_check=True,
                )
            nc.vector.tensor_copy(out=dst[g][:, nsl], in_=ps)
            if alt is not None:
                ae = alt_eng or nc.vector
                ae.tensor_copy(out=alt[g][0:64, nsl], in_=ps[64:128, :])
                ae.tensor_copy(out=alt[g][64:128, nsl], in_=ps[0:64, :])

        def v_chunk(rc):
            rsl = slice(rc * 128, (rc + 1) * 128)
            pst = psA.tile([128, 1024], F32, tag="att", name="projv")
            ps = pst[:, 0:H * HD]
            for c in (2, 0, 1):
                kd = _KDIMS[c]
                nc.tensor.matmul(
                    ps,
                    lhsT=xcT_chunk(c)[:kd, rsl],
                    rhs=wv_sb[c][:kd, :],
                    start=(c == 2),
                    stop=(c == 1),
                    skip_group_check=True,
                )
            nc.vector.tensor_copy(
                out=v_sb[:, rc, :].rearrange("p (h k) -> p h k", h=H)[:, :, 0:HD],
                in_=ps.rearrange("p (h k) -> p h k", h=H),
            )

        # ---------------- attention ----------------
        SCALE = 1.0 / float(np.sqrt(HD))

        class _St:
            pass

        def attn_begin(pair, defer_from=None):
            st = _St()
            st.pair = pair
            st.g = pair // 2
            st.hp0 = (pair % 2) * 2
            pvt = psB.tile([128, 512], F32, tag="pv", name="pv")
            st.pvps = pvt[0:97, :]
            st.prev_pts = None
            st.prev_kg = None
            st.pv_last = None
            st.defer_from = defer_from
            st.deferred = []
            return st

        def _pv_batch(st, kgp, pts, qk_last=None):
            for sub in range(2):
                kc = 2 * kgp + sub
                for a in range(2):
                    h = 2 * st.pair + a
                    mm = nc.tensor.matmul(
                        st.pvps[64 * a:64 * a + 33, :],
                        lhsT=v_sb[:, kc, 33 * h:33 * h + 33],
                        rhs=pts[a][:, sub * 512:(sub + 1) * 512],
                        start=(kc == 0),
                        stop=(kc == N // 128 - 1),
                        tile_position=(0, 64 * a),
                        skip_group_check=True,
                    )
                    if qk_last is not None:
                        add_dep_helper(mm.ins, qk_last.ins, sync=False)
                    st.pv_last = mm

        def attn_block(st, kg, no_alt=False):
            g, hp0 = st.g, st.hp0
            qk_last = None
            pts = None
            if kg < 16:
                attps = [
                    psA.tile([128, 1024], F32, tag="att", name="att")
                    for _ in range(2)
                ]
                for sub in range(2):
                    kc = 2 * kg + sub
                    ksl = slice(kc * 128, (kc + 1) * 128)
                    par = (kc % 2) if not no_alt else 0
                    kt_src = (kT, kTa)[par]
                    qt_src = (qT, qTa)[par]
                    for a in range(2):
                        s = (hp0 + a + 2 * par) % 4
                        mm = nc.tensor.matmul(
                            attps[a][:, sub * 512:(sub + 1) * 512],
                            lhsT=kt_src[g][32 * s:32 * s + 32, ksl],
                            rhs=qt_src[g][32 * s:32 * s + 32, :],
                            start=True,
                            stop=True,
                            tile_position=(32 * s, 0),
                        )
                        if st.pv_last is not None:
                            add_dep_helper(mm.ins, st.pv_last.ins, sync=False)
                        qk_last = mm
                pts = []
                will_defer = st.defer_from is not None and kg >= st.defer_from
                pool = dfp if will_defer else ptp
                for a in range(2):
                    pt = pool.tile([128, 1024], BF16, tag="pt", name="pt")
                    nc.scalar.activation(
                        out=pt,
                        in_=attps[a],
                        func=mybir.ActivationFunctionType.Exp,
                        scale=SCALE,
                    )
                    pts.append(pt)
            if st.prev_pts is not None:
                if st.defer_from is not None and st.prev_kg >= st.defer_from:
                    st.deferred.append((st.prev_kg, st.prev_pts))
                else:
                    _pv_batch(st, st.prev_kg, st.prev_pts, qk_last)
            st.prev_pts = pts
            st.prev_kg = kg if kg < 16 else None

        def attn_flush(st):
            attn_block(st, 16)

        def drain_deferred(st, nbatch=1):
            for _ in range(nbatch):
                if st.deferred:
                    kgp, pts = st.deferred.pop(0)
                    _pv_batch(st, kgp, pts)

        def attn_norm(st, act_recip=False):
            g, hp0 = st.g, st.hp0
            # sums land at rows 0 and 32 (32-aligned); rows in between hold
            # 1.0 so reciprocal stays finite (they hit zero lhsT weights)
            s_sb = stream.tile([2 * HD, 512], F32, tag="s_sb", name="s_sb")
            nc.gpsimd.memset(s_sb, 1.0)
            for a in range(2):
                nc.vector.tensor_scalar_add(
                    out=s_sb[HD * a:HD * a + 1, :],
                    in0=st.pvps[64 * a + 32:64 * a + 33, :],
                    scalar1=1e-6,
                )
            r_sb = stream.tile([2 * HD, 512], F32, tag="r_sb", name="r_sb")
            if act_recip:
                # tail only (ACT idle there): 1/s = exp(-ln(s)), same table
                # set as the exp stream
                lnr = stream.tile([2 * HD, 512], F32, tag="lnr", name="lnr")
                nc.scalar.activation(
                    out=lnr, in_=s_sb, func=mybir.ActivationFunctionType.Ln
                )
                nc.scalar.activation(
                    out=r_sb, in_=lnr,
                    func=mybir.ActivationFunctionType.Exp, scale=-1.0,
                )
            else:
                nc.vector.reciprocal(out=r_sb, in_=s_sb)
            rbct = psA.tile([128, 1024], F32, tag="att", name="rbc")
            rbc = rbct[0:2 * HD, 0:512]
            nc.tensor.matmul(rbc, lhsT=ones2, rhs=r_sb, start=True, stop=True)
            rbc_sb = stream.tile([2 * HD, 512], F32, tag="rbc_sb", name="rbc_sb")
            nc.vector.tensor_copy(out=rbc_sb, in_=rbc)
            for a in range(2):
                nc.vector.tensor_mul(
                    out=catT[g][32 * (hp0 + a):32 * (hp0 + a) + 32, :],
                    in0=st.pvps[64 * a:64 * a + 32, :],
                    in1=rbc_sb[32 * a:32 * a + 32, :],
                )

        # ---------------- emission ----------------
        # critical x DMAs chunked + first so norms start as rows land
        for rc in range(4):
            eng = (nc.sync, nc.scalar)[rc % 2]
            eng.dma_start(
                out=xq_all[:, rc, :], in_=xq[rc * 128:(rc + 1) * 128, :]
            )
        for rc in range(4):
            eng = (nc.sync, nc.scalar, nc.gpsimd)[rc % 3]
            eng.dma_start(
                out=x_all[:, 0, rc, :], in_=x[rc * 128:(rc + 1) * 128, :]
            )
        stgs = load_weights_main()

        squares_act(xq_all, 4, 32)
        inv_lnexp(32, 4)
        squares_act(x_all[:, 0], 4, 0)
        inv_lnexp(0, 4)

        flo = ftp.tile([128, 4, 128], BF16, tag="flo", name="flo")
        fhi = ftp.tile([128, 4, 128], BF16, tag="fhi", name="fhi")
        for rc in range(4):
            scale_mul(rc, flo, fhi, xq_all[:, rc, :], 32 + rc)
        scale_transpose(0, xcq0, xcq1, flo, fhi, eng0=nc.sync, eng1=nc.sync)
        flo = ftp.tile([128, 4, 128], BF16, tag="flo", name="flo")
        fhi = ftp.tile([128, 4, 128], BF16, tag="fhi", name="fhi")
        for rc in range(4):
            scale_mul(rc, flo, fhi, x_all[:, 0, rc, :], rc)
        scale_transpose(0, xcT0, xcT1, flo, fhi, eng0=nc.scalar, eng1=nc.sync)
        load_weights_casts(stgs)
        for g in range(2):
            kq_window(wq_sb, qT, qTa, g, 0, full=False)
        dma_x_window(1, nc.gpsimd)
        dma_x_window(2, nc.gpsimd)
        kq_window(wk_sb, kT, kTa, 0, 0)
        for rc in range(4):
            v_chunk(rc)
        squares_gp(1)

        # -------- build phase: pairs 0+1 stream over windows as built -----
        st0 = attn_begin(0)
        st1 = attn_begin(1, defer_from=DEFER_FROM)
        for nw in range(8):
            wn = nw + 1
            if wn + 2 <= 7:
                dma_x_window(wn + 2, nc.gpsimd)
            if wn + 1 <= 7:
                squares_gp(wn + 1)
            if wn <= 7:
                inv_lnexp(4 * wn, 4)
                flo = ftp.tile([128, 4, 128], BF16, tag="flo", name="flo")
                fhi = ftp.tile([128, 4, 128], BF16, tag="fhi", name="fhi")
                scale_mul(0, flo, fhi, x_all[:, wn % 4, 0, :], 4 * wn + 0)
                scale_mul(1, flo, fhi, x_all[:, wn % 4, 1, :], 4 * wn + 1)
            attn_block(st0, 2 * nw)
            if wn <= 7:
                scale_mul(2, flo, fhi, x_all[:, wn % 4, 2, :], 4 * wn + 2)
                scale_mul(3, flo, fhi, x_all[:, wn % 4, 3, :], 4 * wn + 3)
                scale_transpose(wn, xcT0, xcT1, flo, fhi)
            attn_block(st1, 2 * nw)
            if wn <= 7:
                kq_window(wk_sb, kT, kTa, 0, wn)
            elif nw == 7:
                kq_window(wk_sb, kT, kTa, 1, 0)
            attn_block(st0, 2 * nw + 1)
            if wn <= 7:
                v_chunk(4 * wn + 0)
                v_chunk(4 * wn + 1)
            elif nw == 7:
                kq_window(wk_sb, kT, kTa, 1, 1)
            attn_block(st1, 2 * nw + 1)
            if wn <= 7:
                v_chunk(4 * wn + 2)
                v_chunk(4 * wn + 3)

        attn_flush(st0)
        attn_flush(st1)  # lands in the deferred list

        # -------- phase 2: pairs 2,3; kT g1 + deferred PV drain in-stream --
        st2 = attn_begin(2)
        for kg in range(16):
            attn_block(st2, kg)
            if kg <= 5:
                kq_window(wk_sb, kT, kTa, 1, kg + 2)
            if kg == 1:
                attn_norm(st0)
            if kg in (6, 8, 10, 11, 12):
                drain_deferred(st1, 1)
            if kg == 9:
                load_weights_tail()
            if kg == 13:
                attn_norm(st1)
        st3 = attn_begin(3)
        for kg in range(16):
            attn_block(st3, kg)
            if kg == 1:
                attn_flush(st2)
            if kg == 3:
                attn_norm(st2)
        attn_flush(st3)
        attn_norm(st3, act_recip=True)

        # ---------------- output projection + residual ----------------
        for rw in range(4):
            rsl = slice(rw * 128, (rw + 1) * 128)
            pst = psB.tile([128, 512], F32, tag="pv", name="outp")
            ps = pst[:, 0:IN_DIM]
            for c in range(2):
                nc.tensor.matmul(
                    ps,
                    lhsT=catT[c][:, rsl],
                    rhs=wo_sb[c],
                    start=(c == 0),
                    stop=(c == 1),
                    skip_group_check=True,
                )
            o_sb = stream.tile([128, IN_DIM], F32, tag="o_sb", name="o_sb")
            nc.vector.tensor_add(out=o_sb, in0=ps, in1=xq_all[:, rw, :])
            nc.sync.dma_start(out=out[rsl, :], in_=o_sb)


_NC_CACHE = None


def _get_nc():
    global _NC_CACHE
    if _NC_CACHE is None:
        _NC_CACHE = build_nc()
    return _NC_CACHE


def kernel(_trace=False, **inputs):
    trace = _trace
    x = np.ascontiguousarray(np.asarray(inputs["x"], dtype=np.float32))
    coords = np.asarray(inputs["coords"], dtype=np.int32)
    coordsT = np.ascontiguousarray(coords.T)

    common = {
        "x": x,
        "coordsT": coordsT,
        "wq": np.ascontiguousarray(np.asarray(inputs["wq"], np.float32)),
        "bq": np.ascontiguousarray(np.asarray(inputs["bq"], np.float32)),
        "wk": np.ascontiguousarray(np.asarray(inputs["wk"], np.float32)),
        "bk": np.ascontiguousarray(np.asarray(inputs["bk"], np.float32)),
        "wv": np.ascontiguousarray(np.asarray(inputs["wv"], np.float32)),
        "bv": np.ascontiguousarray(np.asarray(inputs["bv"], np.float32)),
        "wo": np.ascontiguousarray(np.asarray(inputs["wo"], np.float32)),
        "bo": np.ascontiguousarray(np.asarray(inputs["bo"], np.float32)),
    }
    in_maps = []
    for c in range(NC):
        rsl = slice(c * R, (c + 1) * R)
        m = dict(common)
        m["xq"] = np.ascontiguousarray(x[rsl])
        m["cqT"] = np.ascontiguousarray(coordsT[:, rsl])
        in_maps.append(m)

    nc = _get_nc()
    res = run_bass_kernel_spmd(nc, in_maps, list(range(NC)), trace=trace)
    out = np.concatenate([res.results[c]["out"] for c in range(NC)], axis=0)
    if trace:
        return out, res
    return out
